# revision 5
# baseline (speedup 1.0000x reference)
"""Trainium2 Bass kernel for nn_LinearRNN (B=16, T=4096, D_in=256, H=512, D_out=256).

  xp = x @ W_in.T                       [B, T, H]
  h_t = xp_t + h_{t-1} @ W_h.T          (W_h is diagonal -> elementwise scan)
  out = hs @ W_out.T                    [B, T, D_out]

Strategy: batch data-parallel over 8 cores (2 batch rows per core). Per core:
  - host pre-transposes x to [b, d, t] so the contraction dim lands on SBUF
    partitions; weights pre-transposed likewise.
  - matmul1 on TensorE produces xp tiles [h=128, t=512] in PSUM,
  - VectorE tensor_tensor_scan runs the recurrence along the free (t) axis
    with the per-h decay broadcast from a [128,1] column, carry chained
    across t-chunks via the previous tile's last column,
  - matmul2 on TensorE contracts h back to d_out, ScalarE copies PSUM->SBUF,
  - output [b, o, t] DMAs back and the host transposes to [b, t, o].
"""
from contextlib import ExitStack

import numpy as np

import concourse.bass as bass
import concourse.mybir as mybir
import concourse.tile as tile
from concourse import bacc
from concourse.bass_utils import run_bass_kernel_spmd

B, T, D_IN, HID, D_OUT = 16, 4096, 256, 512, 256
NCORES = 8
BPC = B // NCORES          # batch rows per core
TC = 512                   # t-chunk (PSUM bank = 512 fp32)
NCH = T // TC
ND = D_IN // 128           # 2  d-blocks
NH = HID // 128            # 4  h-blocks
NO = D_OUT // 128          # 2  o-blocks
OUT_HALF = T // 2

# 'f32'  : exact fp32 matmuls (4 cyc/row on PE)
# 'f32r' : fp32 storage, PE runs reduced-precision single-pass (1 cyc/row)
# 'bf16' : x/weights/hs cast to bf16 (halves input DMA, fastest PE)
MODE_DEFAULT = "f32r"

_cache: dict = {}


def _build(mode: str) -> bass.Bass:
    f32 = mybir.dt.float32
    dt_in = mybir.dt.bfloat16 if mode == "bf16" else f32
    dt_hs = mybir.dt.bfloat16 if mode == "bf16" else f32

    def mm(ap):
        return ap.bitcast(mybir.dt.float32r) if mode == "f32r" else ap

    nc = bacc.Bacc(None, target_bir_lowering=False)

    xT = nc.declare_dram_parameter("xT", [BPC, D_IN, T], dt_in, isOutput=False)
    w_inT = nc.declare_dram_parameter("w_inT", [D_IN, HID], dt_in, isOutput=False)
    w_outT = nc.declare_dram_parameter("w_outT", [HID, D_OUT], dt_in, isOutput=False)
    dcols = nc.declare_dram_parameter("dcols", [128, NH], f32, isOutput=False)
    out = nc.declare_dram_parameter("out", [BPC, D_OUT, T], f32, isOutput=True)

    with tile.TileContext(nc) as tc, ExitStack() as ctx:
        const_pool = ctx.enter_context(tc.tile_pool(name="const", bufs=1))
        x_pool = ctx.enter_context(tc.tile_pool(name="xt", bufs=2 * ND))
        o_pool = ctx.enter_context(tc.tile_pool(name="ot", bufs=2 * NO * 2))
        hs_pool = ctx.enter_context(tc.tile_pool(name="hs", bufs=12))
        xp_psum = ctx.enter_context(
            tc.tile_pool(name="xp", bufs=4, space=bass.MemorySpace.PSUM))
        op_psum = ctx.enter_context(
            tc.tile_pool(name="op", bufs=3, space=bass.MemorySpace.PSUM))

        wi = []
        for dblk in range(ND):
            w = const_pool.tile([128, HID], dt_in, tag=f"wi{dblk}")
            nc.sync.dma_start(w[:], w_inT[dblk * 128:(dblk + 1) * 128, :])
            wi.append(w)
        wo = []
        for hblk in range(NH):
            w = const_pool.tile([128, D_OUT], dt_in, tag=f"wo{hblk}")
            nc.sync.dma_start(w[:], w_outT[hblk * 128:(hblk + 1) * 128, :])
            wo.append(w)
        dc = const_pool.tile([128, NH], f32, tag="dc")
        nc.sync.dma_start(dc[:], dcols[:])

        for b in range(BPC):
            xt = []
            for dblk in range(ND):
                xtile = x_pool.tile([128, T], dt_in)
                nc.sync.dma_start(xtile[:], xT[b, dblk * 128:(dblk + 1) * 128, :])
                xt.append(xtile)
            ot = {(oblk, half): o_pool.tile([128, OUT_HALF], f32,
                                            name="ot", tag="ot")
                  for oblk in range(NO) for half in range(2)}

            prev_hs = [None] * NH
            for ic in range(NCH):
                tsl = slice(ic * TC, (ic + 1) * TC)
                hs_c = []
                for hblk in range(NH):
                    xp = xp_psum.tile([128, TC], f32)
                    for dblk in range(ND):
                        nc.tensor.matmul(
                            xp[:],
                            mm(wi[dblk][:, hblk * 128:(hblk + 1) * 128]),
                            mm(xt[dblk][:, tsl]),
                            start=(dblk == 0), stop=(dblk == ND - 1))
                    hs = hs_pool.tile([128, TC], dt_hs)
                    init = (0.0 if prev_hs[hblk] is None
                            else prev_hs[hblk][:, TC - 1:TC])
                    nc.vector.tensor_tensor_scan(
                        hs[:], dc[:, hblk:hblk + 1].to_broadcast((128, TC)),
                        xp[:], init,
                        op0=mybir.AluOpType.mult, op1=mybir.AluOpType.add)
                    prev_hs[hblk] = hs
                    hs_c.append(hs)
                half, csl = divmod(ic * TC, OUT_HALF)
                for oblk in range(NO):
                    op = op_psum.tile([128, TC], f32)
                    for hblk in range(NH):
                        nc.tensor.matmul(
                            op[:],
                            mm(wo[hblk][:, oblk * 128:(oblk + 1) * 128]),
                            mm(hs_c[hblk][:]),
                            start=(hblk == 0), stop=(hblk == NH - 1))
                    nc.scalar.copy(ot[(oblk, half)][:, csl:csl + TC], op[:])
                if (ic + 1) * TC % OUT_HALF == 0:
                    for oblk in range(NO):
                        nc.sync.dma_start(
                            out[b, oblk * 128:(oblk + 1) * 128,
                                half * OUT_HALF:(half + 1) * OUT_HALF],
                            ot[(oblk, half)][:])

    nc.compile()
    return nc


def _prep_inputs(x, W_in, W_h, W_out, mode: str):
    npdt = np.float32
    if mode == "bf16":
        import ml_dtypes
        npdt = ml_dtypes.bfloat16
    xT = np.ascontiguousarray(np.transpose(np.asarray(x, np.float32), (0, 2, 1))).astype(npdt)
    w_inT = np.ascontiguousarray(np.asarray(W_in, np.float32).T).astype(npdt)
    w_outT = np.ascontiguousarray(np.asarray(W_out, np.float32).T).astype(npdt)
    d = np.ascontiguousarray(np.diagonal(np.asarray(W_h, np.float32)))
    dcols = np.ascontiguousarray(d.reshape(NH, 128).T, dtype=np.float32)
    in_maps = []
    for c in range(NCORES):
        in_maps.append({
            "xT": np.ascontiguousarray(xT[c * BPC:(c + 1) * BPC]),
            "w_inT": w_inT,
            "w_outT": w_outT,
            "dcols": dcols,
        })
    return in_maps


def _get_nc(mode: str = MODE_DEFAULT):
    if mode not in _cache:
        _cache[mode] = _build(mode)
    return _cache[mode]


def _run(x, W_in, W_h, W_out, mode: str = MODE_DEFAULT, **spmd_kwargs):
    nc = _get_nc(mode)
    in_maps = _prep_inputs(x, W_in, W_h, W_out, mode)
    res = run_bass_kernel_spmd(nc, in_maps, list(range(NCORES)), **spmd_kwargs)
    parts = [np.transpose(np.asarray(res.results[c]["out"]), (0, 2, 1))
             for c in range(NCORES)]
    full = np.concatenate(parts, axis=0).astype(np.float32)
    return full, res


def kernel(x, W_in, W_h, W_out):
    out, _ = _run(x, W_in, W_h, W_out)
    return out
